# revision 28
# baseline (speedup 1.0000x reference)
"""Trainium2 Bass kernel for nn_BiLinearMHSLayer.

Reference computation (per batch element b):
    t  = x @ fc_w.T + fc_b            [S, E]      (S=1024, IN=768, E=256)
    bl = (t @ bi_w.T).reshape(S,L,E) + bias       (L=12)
    out[i,l,j] = sum_e bl[i,l,e] * t[j,e]         [S, L, S]

Sharding: data-parallel over batch B=8 -> one batch element per NeuronCore.

Per-core dataflow (everything kept in "transposed" layout so the contraction
dim lands on SBUF partitions for the PE-array matmuls):
    xT   [IN, S] = PE-transpose of x  (bf16, 48 128x128 tiles)
    tT   [E, S]  = fc_wT.T @ xT  + fc_b          (24 matmuls,  N=512)
    blT  [E*L,S] = bi_wT.T @ tT  + bias          (96 matmuls,  N=512)
    out  (per l) = blT_l.T @ tT                  (384 matmuls, N=512)

The schedule is software-pipelined over S-halves: the first half of xT/tT/blT
unblocks score matmuls + output DMA for i-tiles 0-3 / j-half 0 while the
second half is still being produced, so the 50MB/core output write (the
roofline term) starts early.  PSUM->SBUF evacuation alternates between the
Vector and Scalar engines.  Operands are cast to bf16 (fp32 accumulation in
PSUM); |err| vs the fp32 reference is ~4e-3 of max|out|.
"""

import json

import numpy as np

import concourse.bass as bass
import concourse.mybir as mybir
import concourse.tile as tile
from concourse.bass_utils import run_bass_kernel_spmd

B, S, IN, E, L = 8, 1024, 768, 256, 12
N_CORES = 8
FP32 = mybir.dt.float32
BF16 = mybir.dt.bfloat16
ACT_COPY = mybir.ActivationFunctionType.Copy
ACT_IDENT = mybir.ActivationFunctionType.Identity

# ---------------------------------------------------------------------------
# Workaround: walrus on this image rejects instructions carrying more than one
# embedded sem wait ("Too many sync wait commands", CoreV3GenImpl
# setupSyncWait).  Split excess waits onto EventSemaphore instructions
# inserted immediately before, on the same engine (identical semantics: the
# waits execute, in order, before the instruction).
_WAIT_CAPS = {}
_DEFAULT_WAIT_CAP = 1


def _fix_sync_waits(blob: bytes) -> bytes:
    j = json.loads(blob)
    n = 0
    for f in j.get("functions", []):
        for bb in f.get("blocks", []):
            out = []
            for inst in bb.get("instructions", []):
                si = inst.get("sync_info")
                waits = (si or {}).get("on_wait") or []
                cap = _WAIT_CAPS.get(inst.get("opcode"), _DEFAULT_WAIT_CAP)
                if len(waits) > cap:
                    excess, keep = waits[:-cap], waits[-cap:]
                    for w in excess:
                        n += 1
                        out.append({
                            "debug": inst.get("debug", 0),
                            "engine": inst["engine"],
                            "ins": [],
                            "name": f"waitsplit-{n}",
                            "opcode": "EventSemaphore",
                            "outs": [],
                            "sync_info": {"on_update": [], "on_wait": [w]},
                        })
                    si["on_wait"] = keep
                out.append(inst)
            bb["instructions"] = out
    return json.dumps(j).encode()


# ---------------------------------------------------------------------------
_DMA_SRC_CONST = False  # debug ablation: output DMAs read a constant tile
_SKIP_OUT_DMA = False   # debug ablation: no output DMAs (PE/evac floor)
_EVAC_PAT = "VVA"       # evacuation engine rotation: V=DVE, A=ACT
                        # (Pool/gpsimd cannot access PSUM -- walrus birverifier)
_FUSE_LH = False        # True: one 3.1MB DMA per (i-tile, j-half) unit
_DMA_RINGS = 2          # rotate output stores across SP HWDGE / Pool SWDGE


def _emit_consts(nc, const_pool):
    # memset on DVE so only the (cheap) affine_select occupies the gpsimd
    # queue ahead of the input cast-DMA triggers
    ident = const_pool.tile([128, 128], BF16, tag="ident")
    nc.vector.memset(ident[:], 0.0)
    nc.gpsimd.affine_select(
        out=ident[:], in_=ident[:],
        compare_op=mybir.AluOpType.not_equal, fill=1.0, base=0,
        pattern=[[-1, 128]], channel_multiplier=1)
    stg_const = None
    if _DMA_SRC_CONST:
        stg_const = const_pool.tile([128, 6 * 512], BF16, tag="stg_const")
        nc.vector.memset(stg_const[:], 1.0)
    return ident, stg_const


def _emit_body(nc, tc, pools, dram, ctr, consts):
    """Emit one full per-core computation."""
    x_d, fcw_d, fcb_d, biw_d, bias_d, out_d = dram
    (const_pool, big_pool, in_pool, psum_s, psum_w, stg_pool, dram_pool) = pools
    ident, stg_const = consts

    def evac(dst_ap, src_ap, bias_ap=None, force_act=False):
        """PSUM -> SBUF copy (+ optional per-partition bias add), rotated
        across DVE / ACT / Pool per _EVAC_PAT (weighted by engine rates)."""
        c = ctr[0]
        ctr[0] += 1
        eng = "A" if force_act else _EVAC_PAT[c % len(_EVAC_PAT)]
        if eng == "A":
            if bias_ap is not None:
                # Copy doesn't accept an AP bias; Identity does.
                nc.scalar.activation(dst_ap, src_ap, ACT_IDENT, bias=bias_ap)
            else:
                nc.scalar.activation(dst_ap, src_ap, ACT_COPY)
        else:
            if bias_ap is not None:
                nc.vector.tensor_scalar_add(dst_ap, src_ap, bias_ap)
            else:
                nc.vector.tensor_copy(dst_ap, src_ap)

    # ---- persistent SBUF tensors -------------------------------------------
    x_sb = in_pool.tile([128, 8 * 768], BF16, tag="x_sb")       # [s%128, (s/128, i)]
    fcb_sb = const_pool.tile([128, 2], FP32, tag="fcb_sb")      # col ec: fc_b[ec*128+p]
    bias_sb = const_pool.tile([128, 2], FP32, tag="bias_sb")
    xT = big_pool.tile([128, 6 * 1024], BF16, tag="xT")         # [i%128, (i/128, s)]
    fcwT = big_pool.tile([128, 6 * 256], BF16, tag="fcwT")      # [i%128, (i/128, e)]
    # biwT is fh-major so each f-half's XBAR-transpose destination is one
    # contiguous block: column (ft, kc) -> (ft/12)*3072 + kc*1536 + (ft%12)*128
    biwT = big_pool.tile([128, 2 * 3072], BF16, tag="biwT")
    tT = big_pool.tile([128, 2 * 1024], BF16, tag="tT")         # [e%128, (e/128, s)]
    blT = big_pool.tile([128, 24 * 1024], BF16, tag="blT")      # [f%128, (f/128, s)]

    # ---- input loads --------------------------------------------------------
    # x is cast-loaded to SBUF (PE transposes it -- startup work for the PE).
    # fc_w / bi_w are cast fp32->bf16 into DRAM scratch, then XBAR-transpose
    # DMAs produce fcwT/biwT directly (no PE / PSUM / evac involvement).
    # SWDGE order = startup critical path: x half 0 (gates xT/tT), fc_w
    # (gates fcwT -> tT), bi_w half 0 (gates blT f-tiles 0-11), x half 1.
    fcw_scr = dram_pool.tile([E, IN], BF16, tag="fcw_scr")
    biw_scr = dram_pool.tile([E * L, E], BF16, tag="biw_scr")
    x_src = x_d.rearrange("(n p) i -> p n i", p=128)            # [128, 8, 768]
    x_dst = x_sb[:].rearrange("p (n i) -> p n i", n=8)
    nc.gpsimd.dma_start(out=x_dst[:, 0:2, :], in_=x_src[:, 0:2, :])
    nc.gpsimd.dma_start(out=x_dst[:, 2:4, :], in_=x_src[:, 2:4, :])
    nc.gpsimd.dma_start(out=fcw_scr[:], in_=fcw_d[:])
    nc.gpsimd.dma_start(out=x_dst[:, 4:8, :], in_=x_src[:, 4:8, :])
    nc.gpsimd.dma_start(out=biw_scr[0:1536, :], in_=biw_d[0:1536, :])
    nc.gpsimd.dma_start(out=biw_scr[1536:3072, :], in_=biw_d[1536:3072, :])
    for c in range(2):
        nc.sync.dma_start(out=fcb_sb[:, c:c + 1], in_=fcb_d[c * 128:(c + 1) * 128, :])
        nc.sync.dma_start(out=bias_sb[:, c:c + 1], in_=bias_d[c * 128:(c + 1) * 128, :])
    nc.sync.dma_start_transpose(
        out=fcwT[:].rearrange("p (ic e) -> p ic e", ic=6), in_=fcw_scr[:])
    for fh in range(2):
        nc.sync.dma_start_transpose(
            out=biwT[:, fh * 3072:(fh + 1) * 3072].rearrange(
                "p (kc f) -> p kc f", kc=2),
            in_=biw_scr[fh * 1536:(fh + 1) * 1536, :])

    # ---- building blocks ----------------------------------------------------
    def pe_transpose_group(dst_ap, srcs):
        """Transpose len(srcs) 128x128 blocks into one PSUM bank, evacuate
        with a single wide copy. dst_ap free size must be len(srcs)*128 and
        column-ordered to match srcs."""
        p = psum_s.tile([128, 512], BF16, tag="pms")
        for g, src in enumerate(srcs):
            nc.tensor.transpose(p[:, g * 128:(g + 1) * 128], src, ident[:])
        evac(dst_ap, p[:, 0:len(srcs) * 128])

    def emit_xT(t0, nt):
        # xT columns ic*1024 + n*128 for s-tiles n in [t0, t0+nt)
        for ic in range(6):
            pe_transpose_group(
                xT[:, ic * 1024 + t0 * 128:ic * 1024 + (t0 + nt) * 128],
                [x_sb[:, n * 768 + ic * 128:n * 768 + (ic + 1) * 128]
                 for n in range(t0, t0 + nt)])

    def biwT_col(ft, kc):
        # fh-major biwT layout (see tile declaration above)
        return (ft // 12) * 3072 + kc * 1536 + (ft % 12) * 128

    def emit_tT(ns):
        for ec in range(2):
            p = psum_s.tile([128, 512], FP32, tag="pms")
            for ic in range(6):
                nc.tensor.matmul(
                    p[:],
                    fcwT[:, ic * 256 + ec * 128:ic * 256 + (ec + 1) * 128],
                    xT[:, ic * 1024 + ns * 512:ic * 1024 + (ns + 1) * 512],
                    start=(ic == 0), stop=(ic == 5))
            evac(tT[:, ec * 1024 + ns * 512:ec * 1024 + (ns + 1) * 512],
                 p[:], bias_ap=fcb_sb[:, ec:ec + 1])

    def emit_blT(c0, w, fts=range(24)):
        # one w-wide column sub-block (s in [c0, c0+w)) for f-tiles in fts
        for ft in fts:
            p = psum_s.tile([128, 512], FP32, tag="pms")
            for kc in range(2):
                nc.tensor.matmul(
                    p[:, 0:w],
                    biwT[:, biwT_col(ft, kc):biwT_col(ft, kc) + 128],
                    tT[:, kc * 1024 + c0:kc * 1024 + c0 + w],
                    start=(kc == 0), stop=(kc == 1))
            evac(blT[:, ft * 1024 + c0:ft * 1024 + c0 + w],
                 p[:, 0:w], bias_ap=bias_sb[:, ft % 2:ft % 2 + 1])

    def out_dma(out_ap, in_ap):
        # Rotate output stores across independent descriptor-generation
        # paths (SP HWDGE and the otherwise-idle Pool SWDGE) so trigger /
        # completion handling of consecutive stores proceeds in parallel.
        # ACT is deliberately excluded: a dma trigger's sem-wait executes
        # in-order on the issuing queue and would stall ACT's evac copies.
        if _SKIP_OUT_DMA:
            return
        engines = [nc.sync, nc.gpsimd][:max(1, _DMA_RINGS)]
        eng = engines[ctr[1] % len(engines)]
        ctr[1] += 1
        eng.dma_start(out=out_ap, in_=in_ap)

    def emit_wave(its, lhs=(0, 1), tail_split=False):
        # output unit = (i-tile, l-half) x FULL j: [128 i, 6 l, 1024 j].
        # Full-j units make every partition's DRAM write one contiguous 12KB
        # run -- HW probe showed 2KB-granular strided writes sustain only
        # ~half the bandwidth of contiguous runs.  One l per 2-bank PSUM
        # tile (j-halves in separate banks), single [128,1024] evacuation.
        # tail_split: ship the last unit as two 3-label DMAs so the final
        # drain overlaps the last evacuations.
        for it in its:
            for lh in lhs:
                last = tail_split and it == its[-1] and lh == lhs[-1]
                stg = stg_pool.tile([128, 6 * 1024], BF16, tag="stg")
                for ll in range(6):
                    l = lh * 6 + ll
                    p = psum_w.tile([128, 1024], FP32, tag="pmw")
                    # kc outer: each blT weight tile is loaded once and
                    # streams both j-halves (half the LDWEIGHTS traffic)
                    for kc in range(2):
                        ft = 2 * l + kc
                        for jh in range(2):
                            nc.tensor.matmul(
                                p[:, jh * 512:(jh + 1) * 512],
                                blT[:, ft * 1024 + it * 128:ft * 1024 + (it + 1) * 128],
                                tT[:, kc * 1024 + jh * 512:kc * 1024 + (jh + 1) * 512],
                                start=(kc == 0), stop=(kc == 1))
                    evac(stg[:, ll * 1024:(ll + 1) * 1024], p[:])
                    if last and ll == 2:
                        out_dma(
                            out_d[it * 128:(it + 1) * 128, lh * 6:lh * 6 + 3, :],
                            stg[:, 0:3 * 1024].rearrange("p (l j) -> p l j", l=3))
                if last:
                    out_dma(
                        out_d[it * 128:(it + 1) * 128, lh * 6 + 3:lh * 6 + 6, :],
                        stg[:, 3 * 1024:].rearrange("p (l j) -> p l j", l=3))
                else:
                    out_dma(
                        out_d[it * 128:(it + 1) * 128, lh * 6:lh * 6 + 6, :],
                        stg[:].rearrange("p (l j) -> p l j", l=6))

    # ---- schedule -----------------------------------------------------------
    # blT n-block 0 covers i-tiles 0-3, n-block 1 covers 4-7; tT n-block jh
    # is the j-half.  Waves are ordered so the output DMA stream starts as
    # early as possible and never starves.
    # Full-j output units need both tT halves, so both x/tT halves come
    # first; blT + its weight transposes are still split by l-half so the
    # first units (needing only f-tiles 0-11) ship while f-tiles 12-23 are
    # still being produced.
    emit_xT(0, 2)
    emit_xT(2, 2)
    emit_tT(0)
    emit_xT(4, 4)
    emit_tT(1)
    emit_blT(0, 512, range(0, 12))
    emit_wave((0, 1, 2, 3), lhs=(0,))
    emit_blT(0, 512, range(12, 24))
    emit_wave((0, 1, 2, 3), lhs=(1,))
    emit_blT(512, 512)
    emit_wave((4, 5, 6, 7), tail_split=True)


def build_nc(unroll: int = 1):
    """Build the Bass program.  unroll>1 repeats the whole body (for timing
    measurements via wall-clock differencing)."""
    nc = bass.Bass(trn_type="TRN2")
    x_d = nc.dram_tensor("x", [S, IN], FP32, kind="ExternalInput")
    fcw_d = nc.dram_tensor("fc_w", [E, IN], FP32, kind="ExternalInput")
    fcb_d = nc.dram_tensor("fc_b", [E, 1], FP32, kind="ExternalInput")
    biw_d = nc.dram_tensor("bi_w", [E * L, E], FP32, kind="ExternalInput")
    bias_d = nc.dram_tensor("bias", [E, 1], FP32, kind="ExternalInput")
    # Output is stored bf16 (halves the dominant HBM write stream); the host
    # upcasts to fp32.  Quantization adds ~1e-3 rel err on top of the ~4e-3
    # bf16-compute error -- well inside the 2e-2 gate.
    out_d = nc.dram_tensor("out", [S, L, S], BF16, kind="ExternalOutput")
    dram = (x_d, fcw_d, fcb_d, biw_d, bias_d, out_d)

    with tile.TileContext(nc) as tc:
        with (
            tc.tile_pool(name="const", bufs=1) as const_pool,
            tc.tile_pool(name="big", bufs=1) as big_pool,
            tc.tile_pool(name="inp", bufs=1) as in_pool,
            tc.tile_pool(name="psum_s", bufs=2, space="PSUM") as psum_s,
            tc.tile_pool(name="psum_w", bufs=3, space="PSUM") as psum_w,
            tc.tile_pool(name="stg", bufs=3) as stg_pool,
            tc.tile_pool(name="dram", bufs=1, space="DRAM") as dram_pool,
        ):
            pools = (const_pool, big_pool, in_pool, psum_s, psum_w, stg_pool,
                     dram_pool)
            ctr = [0, 0]
            consts = _emit_consts(nc, const_pool)
            for _ in range(unroll):
                _emit_body(nc, tc, pools, dram, ctr, consts)

    blob = _fix_sync_waits(nc.to_json_bytes())
    nc.to_json_bytes = lambda: blob
    return nc


_CACHE = {}


def _get_nc(unroll: int = 1):
    if unroll not in _CACHE:
        _CACHE[unroll] = build_nc(unroll)
    return _CACHE[unroll]


def kernel(input_tensor, fc_w, fc_b, bi_w, bias):
    input_tensor = np.ascontiguousarray(np.asarray(input_tensor, dtype=np.float32))
    fc_w = np.ascontiguousarray(np.asarray(fc_w, dtype=np.float32))
    fc_b = np.ascontiguousarray(np.asarray(fc_b, dtype=np.float32)).reshape(E, 1)
    bi_w = np.ascontiguousarray(np.asarray(bi_w, dtype=np.float32))
    bias = np.ascontiguousarray(np.asarray(bias, dtype=np.float32)).reshape(E, 1)
    assert input_tensor.shape == (B, S, IN)

    nc = _get_nc()
    in_maps = [
        {"x": input_tensor[c], "fc_w": fc_w, "fc_b": fc_b, "bi_w": bi_w, "bias": bias}
        for c in range(N_CORES)
    ]
    res = run_bass_kernel_spmd(nc, in_maps, core_ids=list(range(N_CORES)))
    return np.stack(
        [np.asarray(res.results[c]["out"]) for c in range(N_CORES)], axis=0
    ).astype(np.float32)



# revision 32
# speedup vs baseline: 1.0436x; 1.0436x over previous
"""Trainium2 Bass kernel for nn_BiLinearMHSLayer.

Reference computation (per batch element b):
    t  = x @ fc_w.T + fc_b            [S, E]      (S=1024, IN=768, E=256)
    bl = (t @ bi_w.T).reshape(S,L,E) + bias       (L=12)
    out[i,l,j] = sum_e bl[i,l,e] * t[j,e]         [S, L, S]

Sharding: data-parallel over batch B=8 -> one batch element per NeuronCore.

Per-core dataflow (everything kept in "transposed" layout so the contraction
dim lands on SBUF partitions for the PE-array matmuls):
    xT   [IN, S] = PE-transpose of x  (bf16, 48 128x128 tiles)
    tT   [E, S]  = fc_wT.T @ xT  + fc_b          (24 matmuls,  N=512)
    blT  [E*L,S] = bi_wT.T @ tT  + bias          (96 matmuls,  N=512)
    out  (per l) = blT_l.T @ tT                  (384 matmuls, N=512)

The schedule is software-pipelined over S-halves: the first half of xT/tT/blT
unblocks score matmuls + output DMA for i-tiles 0-3 / j-half 0 while the
second half is still being produced, so the 50MB/core output write (the
roofline term) starts early.  PSUM->SBUF evacuation alternates between the
Vector and Scalar engines.  Operands are cast to bf16 (fp32 accumulation in
PSUM); |err| vs the fp32 reference is ~4e-3 of max|out|.
"""

import json

import numpy as np

import concourse.bass as bass
import concourse.mybir as mybir
import concourse.tile as tile
from concourse.bass_utils import run_bass_kernel_spmd

B, S, IN, E, L = 8, 1024, 768, 256, 12
N_CORES = 8
FP32 = mybir.dt.float32
BF16 = mybir.dt.bfloat16
ACT_COPY = mybir.ActivationFunctionType.Copy
ACT_IDENT = mybir.ActivationFunctionType.Identity

# ---------------------------------------------------------------------------
# Workaround: walrus on this image rejects instructions carrying more than one
# embedded sem wait ("Too many sync wait commands", CoreV3GenImpl
# setupSyncWait).  Split excess waits onto EventSemaphore instructions
# inserted immediately before, on the same engine (identical semantics: the
# waits execute, in order, before the instruction).
_WAIT_CAPS = {}
_DEFAULT_WAIT_CAP = 1


def _fix_sync_waits(blob: bytes) -> bytes:
    j = json.loads(blob)
    n = 0
    for f in j.get("functions", []):
        for bb in f.get("blocks", []):
            out = []
            for inst in bb.get("instructions", []):
                si = inst.get("sync_info")
                waits = (si or {}).get("on_wait") or []
                cap = _WAIT_CAPS.get(inst.get("opcode"), _DEFAULT_WAIT_CAP)
                if len(waits) > cap:
                    excess, keep = waits[:-cap], waits[-cap:]
                    for w in excess:
                        n += 1
                        out.append({
                            "debug": inst.get("debug", 0),
                            "engine": inst["engine"],
                            "ins": [],
                            "name": f"waitsplit-{n}",
                            "opcode": "EventSemaphore",
                            "outs": [],
                            "sync_info": {"on_update": [], "on_wait": [w]},
                        })
                    si["on_wait"] = keep
                out.append(inst)
            bb["instructions"] = out
    return json.dumps(j).encode()


# ---------------------------------------------------------------------------
_DMA_SRC_CONST = False  # debug ablation: output DMAs read a constant tile
_SKIP_OUT_DMA = False   # debug ablation: no output DMAs (PE/evac floor)
_EVAC_PAT = "VVA"       # evacuation engine rotation: V=DVE, A=ACT
                        # (Pool/gpsimd cannot access PSUM -- walrus birverifier)
_XBAR_W = True          # fcw/biw transposed by XBAR DMA via DRAM scratch
                        # (False: PE transposes, no extra HBM traffic)
_FUSE_LH = False        # True: one 3.1MB DMA per (i-tile, j-half) unit
_DMA_RINGS = 2          # rotate output stores across SP HWDGE / Pool SWDGE


def _emit_consts(nc, const_pool):
    # memset on DVE so only the (cheap) affine_select occupies the gpsimd
    # queue ahead of the input cast-DMA triggers
    ident = const_pool.tile([128, 128], BF16, tag="ident")
    nc.vector.memset(ident[:], 0.0)
    nc.gpsimd.affine_select(
        out=ident[:], in_=ident[:],
        compare_op=mybir.AluOpType.not_equal, fill=1.0, base=0,
        pattern=[[-1, 128]], channel_multiplier=1)
    stg_const = None
    if _DMA_SRC_CONST:
        stg_const = const_pool.tile([128, 6 * 512], BF16, tag="stg_const")
        nc.vector.memset(stg_const[:], 1.0)
    return ident, stg_const


def _emit_body(nc, tc, pools, dram, ctr, consts):
    """Emit one full per-core computation."""
    x_d, fcw_d, fcb_d, biw_d, bias_d, out_d = dram
    (const_pool, big_pool, in_pool, psum_s, psum_w, stg_pool, dram_pool) = pools
    ident, stg_const = consts

    def evac(dst_ap, src_ap, bias_ap=None, force_act=False):
        """PSUM -> SBUF copy (+ optional per-partition bias add), rotated
        across DVE / ACT / Pool per _EVAC_PAT (weighted by engine rates)."""
        c = ctr[0]
        ctr[0] += 1
        eng = "A" if force_act else _EVAC_PAT[c % len(_EVAC_PAT)]
        if eng == "A":
            if bias_ap is not None:
                # Copy doesn't accept an AP bias; Identity does.
                nc.scalar.activation(dst_ap, src_ap, ACT_IDENT, bias=bias_ap)
            else:
                nc.scalar.activation(dst_ap, src_ap, ACT_COPY)
        else:
            if bias_ap is not None:
                nc.vector.tensor_scalar_add(dst_ap, src_ap, bias_ap)
            else:
                nc.vector.tensor_copy(dst_ap, src_ap)

    # ---- persistent SBUF tensors -------------------------------------------
    x_sb = in_pool.tile([128, 8 * 768], BF16, tag="x_sb")       # [s%128, (s/128, i)]
    fcb_sb = const_pool.tile([128, 2], FP32, tag="fcb_sb")      # col ec: fc_b[ec*128+p]
    bias_sb = const_pool.tile([128, 2], FP32, tag="bias_sb")
    xT = big_pool.tile([128, 6 * 1024], BF16, tag="xT")         # [i%128, (i/128, s)]
    fcwT = big_pool.tile([128, 6 * 256], BF16, tag="fcwT")      # [i%128, (i/128, e)]
    # biwT is fh-major so each f-half's XBAR-transpose destination is one
    # contiguous block: column (ft, kc) -> (ft/12)*3072 + kc*1536 + (ft%12)*128
    biwT = big_pool.tile([128, 2 * 3072], BF16, tag="biwT")
    tT = big_pool.tile([128, 2 * 1024], BF16, tag="tT")         # [e%128, (e/128, s)]
    blT = big_pool.tile([128, 24 * 1024], BF16, tag="blT")      # [f%128, (f/128, s)]

    # ---- input loads --------------------------------------------------------
    # x is cast-loaded to SBUF (PE transposes it -- startup work for the PE).
    # _XBAR_W: fc_w / bi_w are cast fp32->bf16 into DRAM scratch, then
    # XBAR-transpose DMAs produce fcwT/biwT directly (no PE / PSUM / evac
    # involvement, but +7MB of HBM traffic for the scratch round trip).
    # Otherwise they are cast-loaded to SBUF and PE-transposed like x.
    x_src = x_d.rearrange("(n p) i -> p n i", p=128)            # [128, 8, 768]
    x_dst = x_sb[:].rearrange("p (n i) -> p n i", n=8)
    nc.gpsimd.dma_start(out=x_dst[:, 0:2, :], in_=x_src[:, 0:2, :])
    nc.gpsimd.dma_start(out=x_dst[:, 2:4, :], in_=x_src[:, 2:4, :])
    if _XBAR_W:
        fcw_scr = dram_pool.tile([E, IN], BF16, tag="fcw_scr")
        biw_scr = dram_pool.tile([E * L, E], BF16, tag="biw_scr")
        nc.gpsimd.dma_start(out=fcw_scr[:], in_=fcw_d[:])
        nc.gpsimd.dma_start(out=x_dst[:, 4:8, :], in_=x_src[:, 4:8, :])
        nc.gpsimd.dma_start(out=biw_scr[0:1536, :], in_=biw_d[0:1536, :])
        nc.gpsimd.dma_start(out=biw_scr[1536:3072, :], in_=biw_d[1536:3072, :])
    else:
        fcw_sb = in_pool.tile([128, 2 * 768], BF16, tag="fcw_sb")
        biw_sb = in_pool.tile([128, 24 * 256], BF16, tag="biw_sb")
        nc.gpsimd.dma_start(
            out=fcw_sb[:].rearrange("p (n i) -> p n i", n=2),
            in_=fcw_d.rearrange("(n p) i -> p n i", p=128))
        nc.gpsimd.dma_start(out=x_dst[:, 4:8, :], in_=x_src[:, 4:8, :])
        nc.gpsimd.dma_start(
            out=biw_sb[:].rearrange("p (n e) -> p n e", n=24),
            in_=biw_d.rearrange("(n p) e -> p n e", p=128))
    for c in range(2):
        nc.sync.dma_start(out=fcb_sb[:, c:c + 1], in_=fcb_d[c * 128:(c + 1) * 128, :])
        nc.sync.dma_start(out=bias_sb[:, c:c + 1], in_=bias_d[c * 128:(c + 1) * 128, :])
    if _XBAR_W:
        nc.sync.dma_start_transpose(
            out=fcwT[:].rearrange("p (ic e) -> p ic e", ic=6), in_=fcw_scr[:])
        for fh in range(2):
            nc.sync.dma_start_transpose(
                out=biwT[:, fh * 3072:(fh + 1) * 3072].rearrange(
                    "p (kc f) -> p kc f", kc=2),
                in_=biw_scr[fh * 1536:(fh + 1) * 1536, :])

    # ---- building blocks ----------------------------------------------------
    def pe_transpose_group(dst_ap, srcs):
        """Transpose len(srcs) 128x128 blocks into one PSUM bank, evacuate
        with a single wide copy. dst_ap free size must be len(srcs)*128 and
        column-ordered to match srcs."""
        p = psum_s.tile([128, 512], BF16, tag="pms")
        for g, src in enumerate(srcs):
            nc.tensor.transpose(p[:, g * 128:(g + 1) * 128], src, ident[:])
        evac(dst_ap, p[:, 0:len(srcs) * 128])

    def emit_xT(t0, nt):
        # xT columns ic*1024 + n*128 for s-tiles n in [t0, t0+nt)
        for ic in range(6):
            pe_transpose_group(
                xT[:, ic * 1024 + t0 * 128:ic * 1024 + (t0 + nt) * 128],
                [x_sb[:, n * 768 + ic * 128:n * 768 + (ic + 1) * 128]
                 for n in range(t0, t0 + nt)])

    def biwT_col(ft, kc):
        if _XBAR_W:  # fh-major layout (see tile declaration above)
            return (ft // 12) * 3072 + kc * 1536 + (ft % 12) * 128
        return kc * 3072 + ft * 128

    def emit_fcwT():
        for ic0 in range(0, 6, 2):
            pe_transpose_group(
                fcwT[:, ic0 * 256:(ic0 + 2) * 256],
                [fcw_sb[:, n * 768 + ic * 128:n * 768 + (ic + 1) * 128]
                 for ic in (ic0, ic0 + 1) for n in (0, 1)])

    def emit_biwT(fts=range(24)):
        # biwT columns kc*3072 + ft*128; group 4 consecutive ft per bank
        for kc in range(2):
            for ft0 in range(fts.start, fts.stop, 4):
                pe_transpose_group(
                    biwT[:, kc * 3072 + ft0 * 128:kc * 3072 + (ft0 + 4) * 128],
                    [biw_sb[:, ft * 256 + kc * 128:ft * 256 + (kc + 1) * 128]
                     for ft in range(ft0, ft0 + 4)])

    def emit_tT(ns):
        for ec in range(2):
            p = psum_s.tile([128, 512], FP32, tag="pms")
            for ic in range(6):
                nc.tensor.matmul(
                    p[:],
                    fcwT[:, ic * 256 + ec * 128:ic * 256 + (ec + 1) * 128],
                    xT[:, ic * 1024 + ns * 512:ic * 1024 + (ns + 1) * 512],
                    start=(ic == 0), stop=(ic == 5))
            evac(tT[:, ec * 1024 + ns * 512:ec * 1024 + (ns + 1) * 512],
                 p[:], bias_ap=fcb_sb[:, ec:ec + 1])

    def emit_blT(c0, w, fts=range(24)):
        # one w-wide column sub-block (s in [c0, c0+w)) for f-tiles in fts
        for ft in fts:
            p = psum_s.tile([128, 512], FP32, tag="pms")
            for kc in range(2):
                nc.tensor.matmul(
                    p[:, 0:w],
                    biwT[:, biwT_col(ft, kc):biwT_col(ft, kc) + 128],
                    tT[:, kc * 1024 + c0:kc * 1024 + c0 + w],
                    start=(kc == 0), stop=(kc == 1))
            evac(blT[:, ft * 1024 + c0:ft * 1024 + c0 + w],
                 p[:, 0:w], bias_ap=bias_sb[:, ft % 2:ft % 2 + 1])

    def out_dma(out_ap, in_ap):
        # Rotate output stores across independent descriptor-generation
        # paths (SP HWDGE and the otherwise-idle Pool SWDGE) so trigger /
        # completion handling of consecutive stores proceeds in parallel.
        # ACT is deliberately excluded: a dma trigger's sem-wait executes
        # in-order on the issuing queue and would stall ACT's evac copies.
        if _SKIP_OUT_DMA:
            return
        engines = [nc.sync, nc.gpsimd][:max(1, _DMA_RINGS)]
        eng = engines[ctr[1] % len(engines)]
        ctr[1] += 1
        eng.dma_start(out=out_ap, in_=in_ap)

    def emit_wave(its, lhs=(0, 1), tail_split=False):
        # output unit = (i-tile, l-half) x FULL j: [128 i, 6 l, 1024 j].
        # Full-j units make every partition's DRAM write one contiguous 12KB
        # run -- HW probe showed 2KB-granular strided writes sustain only
        # ~half the bandwidth of contiguous runs.  One l per 2-bank PSUM
        # tile (j-halves in separate banks), single [128,1024] evacuation.
        # tail_split: ship the last unit as two 3-label DMAs so the final
        # drain overlaps the last evacuations.
        for it in its:
            for lh in lhs:
                last = tail_split and it == its[-1] and lh == lhs[-1]
                stg = stg_pool.tile([128, 6 * 1024], BF16, tag="stg")
                for ll in range(6):
                    l = lh * 6 + ll
                    p = psum_w.tile([128, 1024], FP32, tag="pmw")
                    # kc outer: each blT weight tile is loaded once and
                    # streams both j-halves (half the LDWEIGHTS traffic)
                    for kc in range(2):
                        ft = 2 * l + kc
                        for jh in range(2):
                            nc.tensor.matmul(
                                p[:, jh * 512:(jh + 1) * 512],
                                blT[:, ft * 1024 + it * 128:ft * 1024 + (it + 1) * 128],
                                tT[:, kc * 1024 + jh * 512:kc * 1024 + (jh + 1) * 512],
                                start=(kc == 0), stop=(kc == 1))
                    evac(stg[:, ll * 1024:(ll + 1) * 1024], p[:])
                    if last and ll == 2:
                        out_dma(
                            out_d[it * 128:(it + 1) * 128, lh * 6:lh * 6 + 3, :],
                            stg[:, 0:3 * 1024].rearrange("p (l j) -> p l j", l=3))
                if last:
                    out_dma(
                        out_d[it * 128:(it + 1) * 128, lh * 6 + 3:lh * 6 + 6, :],
                        stg[:, 3 * 1024:].rearrange("p (l j) -> p l j", l=3))
                else:
                    out_dma(
                        out_d[it * 128:(it + 1) * 128, lh * 6:lh * 6 + 6, :],
                        stg[:].rearrange("p (l j) -> p l j", l=6))

    # ---- schedule -----------------------------------------------------------
    # blT n-block 0 covers i-tiles 0-3, n-block 1 covers 4-7; tT n-block jh
    # is the j-half.  Waves are ordered so the output DMA stream starts as
    # early as possible and never starves.
    # Full-j output units need both tT halves, so both x/tT halves come
    # first; blT + its weight transposes are still split by l-half so the
    # first units (needing only f-tiles 0-11) ship while f-tiles 12-23 are
    # still being produced.
    emit_xT(0, 2)
    emit_xT(2, 2)
    if not _XBAR_W:
        emit_fcwT()
    emit_tT(0)
    emit_xT(4, 4)
    emit_tT(1)
    if not _XBAR_W:
        emit_biwT(range(0, 12))
    emit_blT(0, 512, range(0, 12))
    emit_wave((0, 1, 2, 3), lhs=(0,))
    if not _XBAR_W:
        emit_biwT(range(12, 24))
    emit_blT(0, 512, range(12, 24))
    emit_wave((0, 1, 2, 3), lhs=(1,))
    emit_blT(512, 512)
    emit_wave((4, 5, 6, 7), tail_split=True)


def build_nc(unroll: int = 1):
    """Build the Bass program.  unroll>1 repeats the whole body (for timing
    measurements via wall-clock differencing)."""
    nc = bass.Bass(trn_type="TRN2")
    x_d = nc.dram_tensor("x", [S, IN], FP32, kind="ExternalInput")
    fcw_d = nc.dram_tensor("fc_w", [E, IN], FP32, kind="ExternalInput")
    fcb_d = nc.dram_tensor("fc_b", [E, 1], FP32, kind="ExternalInput")
    biw_d = nc.dram_tensor("bi_w", [E * L, E], FP32, kind="ExternalInput")
    bias_d = nc.dram_tensor("bias", [E, 1], FP32, kind="ExternalInput")
    # Output is stored bf16 (halves the dominant HBM write stream); the host
    # upcasts to fp32.  Quantization adds ~1e-3 rel err on top of the ~4e-3
    # bf16-compute error -- well inside the 2e-2 gate.
    out_d = nc.dram_tensor("out", [S, L, S], BF16, kind="ExternalOutput")
    dram = (x_d, fcw_d, fcb_d, biw_d, bias_d, out_d)

    with tile.TileContext(nc) as tc:
        with (
            tc.tile_pool(name="const", bufs=1) as const_pool,
            tc.tile_pool(name="big", bufs=1) as big_pool,
            tc.tile_pool(name="inp", bufs=1) as in_pool,
            tc.tile_pool(name="psum_s", bufs=2, space="PSUM") as psum_s,
            tc.tile_pool(name="psum_w", bufs=3, space="PSUM") as psum_w,
            tc.tile_pool(name="stg", bufs=3) as stg_pool,
            tc.tile_pool(name="dram", bufs=1, space="DRAM") as dram_pool,
        ):
            pools = (const_pool, big_pool, in_pool, psum_s, psum_w, stg_pool,
                     dram_pool)
            ctr = [0, 0]
            consts = _emit_consts(nc, const_pool)
            for _ in range(unroll):
                _emit_body(nc, tc, pools, dram, ctr, consts)

    blob = _fix_sync_waits(nc.to_json_bytes())
    nc.to_json_bytes = lambda: blob
    return nc


_CACHE = {}


def _get_nc(unroll: int = 1):
    if unroll not in _CACHE:
        _CACHE[unroll] = build_nc(unroll)
    return _CACHE[unroll]


def kernel(input_tensor, fc_w, fc_b, bi_w, bias):
    input_tensor = np.ascontiguousarray(np.asarray(input_tensor, dtype=np.float32))
    fc_w = np.ascontiguousarray(np.asarray(fc_w, dtype=np.float32))
    fc_b = np.ascontiguousarray(np.asarray(fc_b, dtype=np.float32)).reshape(E, 1)
    bi_w = np.ascontiguousarray(np.asarray(bi_w, dtype=np.float32))
    bias = np.ascontiguousarray(np.asarray(bias, dtype=np.float32)).reshape(E, 1)
    assert input_tensor.shape == (B, S, IN)

    nc = _get_nc()
    in_maps = [
        {"x": input_tensor[c], "fc_w": fc_w, "fc_b": fc_b, "bi_w": bi_w, "bias": bias}
        for c in range(N_CORES)
    ]
    res = run_bass_kernel_spmd(nc, in_maps, core_ids=list(range(N_CORES)))
    return np.stack(
        [np.asarray(res.results[c]["out"]) for c in range(N_CORES)], axis=0
    ).astype(np.float32)

